# revision 1
# baseline (speedup 1.0000x reference)
"""DeepSeek-V3 MLA attention (B=1, S=2048) on 8 TRN2 NeuronCores.

v3 sharding: low-rank down-projections are token-sharded (256
tokens/core).  Each core computes the q up-projection for ALL 32 heads
on its own token chunk; four single-head AllToAlls redistribute q so
each core ends up with its 4 heads over all 2048 tokens, pipelined
against per-head attention.  The kv latents (512 raw + 64 roped k_rot
+ 1 bf16 rstd row) are AllGathered early; consumers scale the latents
by rstd.  Attention and o_proj are head-sharded; each core returns a
partial o_proj output which the host sums.

All activations/weights are bf16 (PSUM stays fp32).  Weights are
host-pretiled so every DMA moves >=512B contiguous runs.  The
attention inner loop computes scores two key-tiles ahead of the AV
accumulation to hide the exp (scalar engine) latency.
"""

import numpy as np
import ml_dtypes

import concourse.bass as bass
import concourse.tile as tile
from concourse import bacc, mybir
from concourse.bass_utils import run_bass_kernel_spmd

F32 = mybir.dt.float32
BF16 = mybir.dt.bfloat16
NPBF16 = ml_dtypes.bfloat16
AF = mybir.ActivationFunctionType

HIDDEN = 4096
N_HEADS = 32
Q_LORA = 1536
KV_LORA = 512
ROPE_D = 64
NOPE_D = 128
V_D = 128
QH = NOPE_D + ROPE_D  # 192
EPS = 1e-6
SCALING = QH ** -0.5
S = 2048
NCORE = 8
SC = S // NCORE  # 256 tokens per core chunk
HPC = N_HEADS // NCORE  # 4 heads per core

QKT = Q_LORA // 128  # 12
KVKT = KV_LORA // 128  # 4
HKT = HIDDEN // 128  # 32
NGRP = 4  # q AllToAll groups (1 local head each)
GT = 12  # m-tiles per group: 8 nope (per dest) + 4 packed rot pairs
KVR = KV_LORA + ROPE_D + 1  # 577 rows in the kv gather (incl rstd row)
BLK = 192  # a2a block rows per dest: nope 128 + rot 64


def _qs(qb):
    return slice(qb * 512, (qb + 1) * 512)


def _kts(kt):
    return slice(kt * 128, (kt + 1) * 128)


def build():
    nc = bacc.Bacc(None, target_bir_lowering=False, num_devices=NCORE)

    def din(name, shape, dt=BF16):
        return nc.dram_tensor(name, shape, dt, kind="ExternalInput")

    # host-pretiled weights: row p of each [128, X] block is the SBUF
    # partition row (contiguous >=512B runs for every DMA)
    ht_in = din("ht_in", [128, HKT * SC])              # hidden own chunk
    wkva_t = din("wkva_t", [128, KVKT * HIDDEN])       # kv_a m-tiles
    wkr_t = din("wkr_t", [128, HKT * ROPE_D])          # k_rot tile
    wqa_t = din("wqa_t", [128, QKT * HIDDEN])          # q_a m-tiles
    wqb_t = din("wqb_t", [128, NGRP * GT * Q_LORA])    # q_b m-tiles (all heads)
    wkb_t = din("wkb_t", [128, KVKT * HPC * 128])      # k_nope up (own heads)
    wvb_t = din("wvb_t", [128, KVKT * HPC * 128])      # v up (own heads)
    wo_t = din("wo_t", [128, HPC * HIDDEN])            # o_proj (own heads)
    cos_kc = din("cos_kc", [ROPE_D, SC])               # own-chunk cos/sin
    sin_kc = din("sin_kc", [ROPE_D, SC])
    cos_q = din("cos_q", [2 * ROPE_D, SC])             # duplicated halves
    sin_q = din("sin_q", [2 * ROPE_D, SC])
    rot64 = din("rot64", [ROPE_D, ROPE_D])
    rot128 = din("rot128", [2 * ROPE_D, 2 * ROPE_D])
    mask_strip = din("mask_strip", [128, 896])
    ones_bf = din("ones_bf", [128, 1])
    eps_in = din("eps_in", [1, 1], F32)

    o_partial = nc.dram_tensor("o_partial", [HIDDEN, S], BF16, kind="ExternalOutput")

    ag_in_kv = nc.dram_tensor("ag_in_kv", [KVR, SC], BF16, kind="Internal")
    ag_out_kv = nc.dram_tensor(
        "ag_out_kv", [NCORE * KVR, SC], BF16, kind="Internal", addr_space="Shared"
    )
    a2a_in = [
        nc.dram_tensor(f"a2a_in{g}", [NCORE * BLK, SC], BF16, kind="Internal")
        for g in range(NGRP)
    ]
    a2a_out = [
        nc.dram_tensor(f"a2a_out{g}", [NCORE * BLK, SC], BF16, kind="Internal")
        for g in range(NGRP)
    ]

    with tile.TileContext(nc) as tc:
      with tc.tile_pool(name="root", bufs=1) as R:
        ones1 = R.tile([128, 1], BF16)
        nc.scalar.dma_start(ones1[:], ones_bf[:])
        epst = R.tile([1, 1], F32)
        nc.scalar.dma_start(epst[:], eps_in[:])
        # long-lived phase-2 tiles allocated up front so their DMAs can run
        # while phase 1 computes (they do not overlap phase-1 SBUF pools)
        wkb = R.tile([128, KVKT, HPC * 128], BF16)
        wvb = R.tile([128, KVKT, HPC * 128], BF16)
        wo_sb = R.tile([128, HPC, HIDDEN], BF16)
        mask_sb = R.tile([128, 896], BF16)
        lat = R.tile([128, KVKT, S], BF16)
        k_rot = R.tile([ROPE_D, S], BF16)
        ssq_row = R.tile([1, S], BF16)
        with (
            tc.tile_pool(name="glob", bufs=1) as G,
            tc.tile_pool(name="ph1", bufs=1) as P1,
            tc.tile_pool(name="wstream", bufs=1) as WS,
            tc.tile_pool(name="ps_lat", bufs=4, space="PSUM") as PSL,
            tc.tile_pool(name="ps_acc", bufs=1, space="PSUM") as PSA,
        ):
            # ---- loads: first kv_a weight tile, then hidden, then the rest
            wkva_r = wkva_t[:].rearrange("p (m k n) -> p m k n", m=KVKT, k=HKT)
            wkva_tiles = []
            for m in range(2):
                wt = WS.tile([128, HKT, 128], BF16, tag="wkva", bufs=2)
                nc.sync.dma_start(wt[:], wkva_r[:, m])
                wkva_tiles.append(wt)
            ht = G.tile([128, HKT, SC], BF16)
            ht_r = ht_in[:].rearrange("p (k n) -> p k n", k=HKT)
            for q4 in range(4):
                nc.sync.dma_start(
                    ht[:, q4 * 8 : (q4 + 1) * 8, :], ht_r[:, q4 * 8 : (q4 + 1) * 8]
                )
            r64 = P1.tile([ROPE_D, ROPE_D], BF16)
            nc.scalar.dma_start(r64[:], rot64[:])
            cckc = P1.tile([ROPE_D, SC], BF16)
            sckc = P1.tile([ROPE_D, SC], BF16)
            nc.scalar.dma_start(cckc[:], cos_kc[:])
            nc.scalar.dma_start(sckc[:], sin_kc[:])
            r128 = P1.tile([2 * ROPE_D, 2 * ROPE_D], BF16)
            nc.scalar.dma_start(r128[:], rot128[:])
            ccq = P1.tile([2 * ROPE_D, SC], BF16)
            scq = P1.tile([2 * ROPE_D, SC], BF16)
            nc.scalar.dma_start(ccq[:], cos_q[:])
            nc.scalar.dma_start(scq[:], sin_q[:])

            # ================= phase 1a: kv latents (own chunk) =========
            # raw latents + roped k_rot + bf16 rstd row; normalize on the
            # consumer side after the AllGather.
            kvraw = P1.tile([128, KVKT, SC], BF16)
            sskv = PSA.tile([1, SC], F32, name="sskv")
            kv_sq = []
            for m in range(KVKT):
                ps = PSL.tile([128, SC], F32, tag="pslat")
                if m < 2:
                    wt = wkva_tiles[m]
                else:
                    wt = WS.tile([128, HKT, 128], BF16, tag="wkva", bufs=2)
                    nc.sync.dma_start(wt[:], wkva_r[:, m])
                for k in range(HKT):
                    nc.tensor.matmul(
                        ps[:], wt[:, k, :], ht[:, k, :], start=(k == 0), stop=(k == HKT - 1)
                    )
                nc.vector.tensor_copy(kvraw[:, m, :], ps[:])
                sq = WS.tile([128, SC], BF16, tag="lsq", bufs=3)
                nc.scalar.activation(sq[:], ps[:], AF.Square)
                kv_sq.append(sq)
                if m >= 1:  # deferred ssq matmul: PE never waits on Act
                    nc.tensor.matmul(
                        sskv[:], ones1[:], kv_sq[m - 1][:], start=(m == 1), stop=False
                    )
            nc.tensor.matmul(sskv[:], ones1[:], kv_sq[-1][:], start=False, stop=True)
            nc.gpsimd.dma_start(
                ag_in_kv[0:KV_LORA, :].rearrange("(m p) n -> p m n", p=128), kvraw[:]
            )

            # k_rot (rows 512:576 of ckv), roped with own chunk cos/sin
            psr = PSL.tile([ROPE_D, SC], F32, tag="psrot", bufs=2)
            wtr = WS.tile([128, HKT, ROPE_D], BF16, tag="wkr", bufs=1)
            nc.sync.dma_start(wtr[:], wkr_t[:].rearrange("p (k n) -> p k n", k=HKT))
            for k in range(HKT):
                nc.tensor.matmul(
                    psr[:], wtr[:, k, :], ht[:, k, :], start=(k == 0), stop=(k == HKT - 1)
                )
            kraw = P1.tile([ROPE_D, SC], BF16)
            nc.vector.tensor_copy(kraw[:], psr[:])
            rps = PSL.tile([ROPE_D, SC], F32, tag="psrot", bufs=2)
            nc.tensor.matmul(rps[:], r64[:], kraw[:], start=True, stop=True)
            ktmp = P1.tile([ROPE_D, SC], BF16)
            nc.vector.tensor_mul(ktmp[:], rps[:], sckc[:])
            kfin = P1.tile([ROPE_D, SC], BF16)
            nc.vector.tensor_mul(kfin[:], kraw[:], cckc[:])
            nc.vector.tensor_add(kfin[:], kfin[:], ktmp[:])
            nc.gpsimd.dma_start(ag_in_kv[KV_LORA : KV_LORA + ROPE_D, :], kfin[:])

            # ship the raw sum-of-squares row; consumers do sqrt+recip
            sskv_bf = P1.tile([1, SC], BF16)
            nc.vector.tensor_copy(sskv_bf[:], sskv[:])
            nc.gpsimd.dma_start(ag_in_kv[KV_LORA + ROPE_D : KVR, :], sskv_bf[:])
            nc.gpsimd.collective_compute(
                "AllGather",
                mybir.AluOpType.bypass,
                replica_groups=[list(range(NCORE))],
                ins=[ag_in_kv[:]],
                outs=[ag_out_kv[:]],
            )

            # ================= phase 1b: q latents (own chunk) ==========
            qraw = P1.tile([128, QKT, SC], BF16)
            ssq = PSA.tile([1, SC], F32, name="ssq")
            wqa_r = wqa_t[:].rearrange("p (m k n) -> p m k n", m=QKT, k=HKT)
            q_sq = []
            for m in range(QKT):
                ps = PSL.tile([128, SC], F32, tag="pslat")
                wt = WS.tile([128, HKT, 128], BF16, tag="wqa", bufs=4)
                nc.sync.dma_start(wt[:], wqa_r[:, m])
                for k in range(HKT):
                    nc.tensor.matmul(
                        ps[:], wt[:, k, :], ht[:, k, :], start=(k == 0), stop=(k == HKT - 1)
                    )
                nc.vector.tensor_copy(qraw[:, m, :], ps[:])
                sq = WS.tile([128, SC], BF16, tag="lsq", bufs=3)
                nc.scalar.activation(sq[:], ps[:], AF.Square)
                q_sq.append(sq)
                if m >= 1:
                    nc.tensor.matmul(
                        ssq[:], ones1[:], q_sq[m - 1][:], start=(m == 1), stop=False
                    )
            nc.tensor.matmul(ssq[:], ones1[:], q_sq[-1][:], start=False, stop=True)

            nc.sync.dma_start(wkb[:], wkb_t[:].rearrange("p (k n) -> p k n", k=KVKT))
            nc.sync.dma_start(wvb[:], wvb_t[:].rearrange("p (k n) -> p k n", k=KVKT))
            nc.sync.dma_start(mask_sb[:], mask_strip[:])

            sq1 = P1.tile([1, SC], F32)
            nc.scalar.activation(sq1[:], ssq[:], AF.Sqrt, scale=1.0 / Q_LORA, bias=epst[:])
            rq = P1.tile([1, SC], F32)
            nc.vector.reciprocal(rq[:], sq1[:])
            rq_bc = P1.tile([128, SC], F32)
            nc.gpsimd.partition_broadcast(rq_bc[:], rq[:])
            ql = P1.tile([128, QKT, SC], BF16)
            for m in range(QKT):
                nc.vector.tensor_mul(ql[:, m, :], qraw[:, m, :], rq_bc[:])

            # ================= phase 1c: q_b all heads (own chunk) ======
            # m-tiles per group g (head g of each dest): 8 nope tiles
            # (dest-major) then 4 rot tiles packing dests (2j | 2j+1).
            wqb_r = wqb_t[:].rearrange("p (t k n) -> p t k n", t=NGRP * GT, k=QKT)
            qown = [G.tile([128, GT, SC], BF16, name=f"qown{g}") for g in range(NGRP)]
            for g in range(NGRP):
                for mt in range(GT):
                    ps = PSL.tile([128, SC], F32, tag="pslat")
                    wt = WS.tile([128, QKT, 128], BF16, tag="wqb", bufs=6)
                    nc.sync.dma_start(wt[:], wqb_r[:, g * GT + mt])
                    for k in range(QKT):
                        nc.tensor.matmul(
                            ps[:], wt[:, k, :], ql[:, k, :], start=(k == 0), stop=(k == QKT - 1)
                        )
                    if mt % 2 == 0:
                        nc.vector.tensor_copy(qown[g][:, mt, :], ps[:])
                    else:
                        nc.scalar.copy(qown[g][:, mt, :], ps[:])
                # rope the 4 rot-pair tiles (mt = 8..11)
                for j in range(4):
                    rtile = qown[g][:, 8 + j, :]
                    rp2 = PSL.tile([128, SC], F32, tag="pslat")
                    nc.tensor.matmul(rp2[:], r128[:], rtile, start=True, stop=True)
                    rtmp = WS.tile([128, SC], BF16, tag="rtmp", bufs=2)
                    nc.vector.tensor_mul(rtmp[:], rp2[:], scq[:])
                    nc.vector.tensor_mul(rtile, rtile, ccq[:])
                    nc.vector.tensor_add(rtile, rtile, rtmp[:])
                # ship: nope tile d -> block rows d*192..+128; rot tile j
                # halves -> rows (2j)*192+128 and (2j+1)*192+128
                a2a_nope = a2a_in[g][:].rearrange("(d b) n -> b d n", d=NCORE)
                nc.gpsimd.dma_start(a2a_nope[0:128], qown[g][:, 0:8, :])
                a2a_rot = a2a_in[g][:].rearrange("(j b) n -> b j n", j=4)
                nc.gpsimd.dma_start(a2a_rot[128:192], qown[g][0:64, 8:12, :])
                nc.gpsimd.dma_start(a2a_rot[320:384], qown[g][64:128, 8:12, :])
                nc.gpsimd.collective_compute(
                    "AllToAll",
                    mybir.AluOpType.bypass,
                    replica_groups=[list(range(NCORE))],
                    ins=[a2a_in[g][:]],
                    outs=[a2a_out[g][:]],
                )

            # o_proj weights: DMA engines are free once the q_b stream ends
            with tc.tile_wait_until(0.108):
                nc.sync.dma_start(
                    wo_sb[:], wo_t[:].rearrange("p (h n) -> p h n", h=HPC)
                )

        # ================= phase 2: head-sharded attention ==============
        with tc.tile_pool(name="p2", bufs=1) as P2:
            # hold these back in scheduler time so the kvAG-dependent DMAs do
            # not head-block the engine queues during phase 1
            with tc.tile_wait_until(0.108):
                # kv latents (raw) + k_rot + ssq row (after kv AG)
                ag_rcn = ag_out_kv[:].rearrange("(c r) n -> r c n", c=NCORE)
                for m in range(KVKT):
                    nc.scalar.dma_start(
                        lat[:, m, :].rearrange("p (c n) -> p c n", c=NCORE),
                        ag_rcn[m * 128 : (m + 1) * 128],
                    )
                nc.scalar.dma_start(
                    k_rot[:].rearrange("p (c n) -> p c n", c=NCORE),
                    ag_rcn[KV_LORA : KV_LORA + ROPE_D],
                )
                nc.scalar.dma_start(
                    ssq_row[:].rearrange("p (c n) -> p c n", c=NCORE),
                    ag_out_kv[:].rearrange("(c r) n -> r c n", c=NCORE)[
                        KV_LORA + ROPE_D : KVR
                    ],
                )
                sq2r = P2.tile([1, S], F32)
                nc.scalar.activation(
                    sq2r[:], ssq_row[:], AF.Sqrt, scale=1.0 / KV_LORA, bias=epst[:]
                )
                rkv_row = P2.tile([1, S], BF16)
                with nc.allow_low_precision(reason="bf16 rstd scale on bf16 latents"):
                    nc.vector.reciprocal(rkv_row[:], sq2r[:])
                rkv_bc = P2.tile([128, S], BF16)
                nc.gpsimd.partition_broadcast(rkv_bc[:], rkv_row[:])
                for m in range(KVKT):
                    nc.vector.tensor_mul(lat[:, m, :], lat[:, m, :], rkv_bc[:])

            # kv_b: kn (4 heads) and v (all tokens)
            kn = P2.tile([128, HPC, S], BF16)
            vp = P2.tile([128, 16, HPC * 128], BF16)
            with (
                tc.tile_pool(name="ps2", bufs=4, space="PSUM") as PSK,
                tc.tile_pool(name="ps2o", bufs=2, space="PSUM") as PAO,
                tc.tile_pool(name="ps2s", bufs=2, space="PSUM") as PSS,
            ):
                for h in range(HPC):
                    for cb in range(4):
                        pk = PSK.tile([128, 512], F32, tag="sps")
                        for k in range(KVKT):
                            nc.tensor.matmul(
                                pk[:],
                                wkb[:, k, h * 128 : (h + 1) * 128],
                                lat[:, k, _qs(cb)],
                                start=(k == 0),
                                stop=(k == KVKT - 1),
                            )
                        if cb % 2 == 0:
                            nc.vector.tensor_copy(kn[:, h, _qs(cb)], pk[:])
                        else:
                            nc.scalar.copy(kn[:, h, _qs(cb)], pk[:])
                for tt in range(16):
                    pv = PSK.tile([128, 512], F32, tag="sps")
                    for k in range(KVKT):
                        nc.tensor.matmul(
                            pv[:],
                            lat[:, k, _kts(tt)],
                            wvb[:, k, :],
                            start=(k == 0),
                            stop=(k == KVKT - 1),
                        )
                    if tt % 2 == 0:
                        nc.vector.tensor_copy(vp[:, tt, :], pv[:])
                    else:
                        nc.scalar.copy(vp[:, tt, :], pv[:])

                o_sb = P2.tile([128, HPC, S], BF16)

                # attention per group (1 head), pipelined against its A2A
                for g in range(NGRP):
                    h = g
                    a2a_r = a2a_out[g][:].rearrange("(c b) n -> b c n", c=NCORE)
                    qn = P2.tile([128, S], BF16, tag="qn", bufs=2)
                    qr = P2.tile([ROPE_D, S], BF16, tag="qr", bufs=2)
                    with tc.tile_wait_until(0.146 + 0.0347 * g):
                        for cp in range(4):  # 512-token pieces, in qb order
                            nc.sync.dma_start(
                                qn[:, _qs(cp)].rearrange("p (c n) -> p c n", c=2),
                                a2a_r[0:128, 2 * cp : 2 * cp + 2],
                            )
                            nc.sync.dma_start(
                                qr[:, _qs(cp)].rearrange("p (c n) -> p c n", c=2),
                                a2a_r[128:192, 2 * cp : 2 * cp + 2],
                            )
                    for qb in range(4):
                        kt_max = 4 * (qb + 1)
                        ops = PAO.tile([128, 512], F32, tag="ops")
                        ssum = PSS.tile([1, 512], F32, tag="ssum")

                        def mk_sps(kt):
                            sps = PSK.tile([128, 512], F32, tag="sps")
                            nc.tensor.matmul(
                                sps[:],
                                kn[:, h, _kts(kt)],
                                qn[:, _qs(qb)],
                                start=True,
                                stop=False,
                            )
                            nc.tensor.matmul(
                                sps[:],
                                k_rot[:, _kts(kt)],
                                qr[:, _qs(qb)],
                                start=False,
                                stop=True,
                            )
                            return sps

                        PF = 2  # score tiles computed ahead of AV/ssum
                        queue = [mk_sps(kt) for kt in range(min(PF, kt_max))]
                        for kt in range(kt_max):
                            if kt + PF < kt_max:
                                queue.append(mk_sps(kt + PF))
                            sps = queue.pop(0)
                            pt = P2.tile([128, 512], BF16, tag="pt", bufs=5)
                            nc.scalar.activation(pt[:], sps[:], AF.Exp, scale=SCALING)
                            v_idx = kt - 4 * qb
                            if v_idx >= 0:
                                nc.vector.tensor_mul(
                                    pt[:],
                                    pt[:],
                                    mask_sb[:, 384 - v_idx * 128 : 896 - v_idx * 128],
                                )
                            nc.tensor.matmul(
                                ops[:],
                                vp[:, kt, h * 128 : (h + 1) * 128],
                                pt[:],
                                start=(kt == 0),
                                stop=(kt == kt_max - 1),
                            )
                            nc.tensor.matmul(
                                ssum[:],
                                ones1[:],
                                pt[:],
                                start=(kt == 0),
                                stop=(kt == kt_max - 1),
                            )
                        rec = P2.tile([1, 512], F32, tag="rec", bufs=2)
                        nc.vector.reciprocal(rec[:], ssum[:])
                        rb = P2.tile([128, 512], F32, tag="rb", bufs=2)
                        nc.gpsimd.partition_broadcast(rb[:], rec[:])
                        nc.vector.tensor_mul(o_sb[:, h, _qs(qb)], ops[:], rb[:])

                # ---------------- o_proj --------------------------------
                for m in range(HIDDEN // 128):
                    ost = P2.tile([128, 4, 512], BF16, tag="ost", bufs=2)
                    for qb in range(4):
                        po = PSK.tile([128, 512], F32, tag="sps")
                        for hh in range(HPC):
                            nc.tensor.matmul(
                                po[:],
                                wo_sb[:, hh, m * 128 : (m + 1) * 128],
                                o_sb[:, hh, _qs(qb)],
                                start=(hh == 0),
                                stop=(hh == HPC - 1),
                            )
                        if qb % 2 == 0:
                            nc.vector.tensor_copy(ost[:, qb, :], po[:])
                        else:
                            nc.scalar.copy(ost[:, qb, :], po[:])
                    nc.sync.dma_start(
                        o_partial[m * 128 : (m + 1) * 128, :],
                        ost[:].rearrange("p a b -> p (a b)"),
                    )

    nc.compile()
    return nc


def _tile_lhsT(wT, n_k, n_m):
    """wT: [K, M] (lhsT layout, K contracting).  Returns pretiled
    [128, n_m*n_k*128]: A[p, m, k, c] = wT[k*128+p, m*128+c]."""
    K, M = wT.shape
    assert K == n_k * 128 and M == n_m * 128
    A = wT.reshape(n_k, 128, n_m, 128).transpose(1, 2, 0, 3)  # [p, m, k, c]
    return np.ascontiguousarray(A.reshape(128, n_m * n_k * 128))


def make_in_maps(
    hidden_states, cos, sin, w_q_a, q_a_ln_w, w_q_b, w_kv_a, kv_a_ln_w, w_kv_b, w_o
):
    f32 = np.float32
    bf = NPBF16
    hidden_T = np.asarray(hidden_states, dtype=f32)[0].T  # [4096, 2048]
    cos_T = np.asarray(cos, dtype=f32)[0].T  # [64, 2048]
    sin_T = np.asarray(sin, dtype=f32)[0].T

    w_q_a = np.asarray(w_q_a, dtype=f32)
    w_kv_a = np.asarray(w_kv_a, dtype=f32)
    w_qb_eff = np.asarray(w_q_b, dtype=f32) * np.asarray(q_a_ln_w, dtype=f32)[None, :]
    w_kvb_eff = np.asarray(w_kv_b, dtype=f32) * np.asarray(kv_a_ln_w, dtype=f32)[None, :]
    w_o = np.asarray(w_o, dtype=f32)

    wqa_t = _tile_lhsT(w_q_a.T.astype(bf), HKT, QKT)
    wkva_t = _tile_lhsT(w_kv_a[:KV_LORA].T.astype(bf), HKT, KVKT)
    wkr = w_kv_a[KV_LORA:].T.astype(bf)  # [4096, 64]
    wkr_t = np.ascontiguousarray(
        wkr.reshape(HKT, 128, ROPE_D).transpose(1, 0, 2).reshape(128, HKT * ROPE_D)
    )

    # q_b m-tiles: group g (local head g): 8 nope tiles (dest-major), then
    # 4 rot tiles packing dests (2j | 2j+1)
    wqbT = w_qb_eff.T.astype(bf)  # [1536, 6144]
    mtiles = []
    for g in range(NGRP):
        for d in range(NCORE):
            hd = 4 * d + g
            mtiles.append(wqbT[:, hd * QH : hd * QH + NOPE_D])
        for j in range(4):
            h0 = 4 * (2 * j) + g
            h1 = 4 * (2 * j + 1) + g
            rp = np.concatenate(
                [
                    wqbT[:, h0 * QH + NOPE_D : (h0 + 1) * QH],
                    wqbT[:, h1 * QH + NOPE_D : (h1 + 1) * QH],
                ],
                axis=1,
            )
            mtiles.append(rp)
    wqb_all = np.concatenate(mtiles, axis=1)  # [1536, 48*128]
    wqb_t = _tile_lhsT(wqb_all, QKT, NGRP * GT)

    rot = np.zeros((ROPE_D, ROPE_D), dtype=f32)
    half = ROPE_D // 2
    rot[np.arange(half), np.arange(half) + half] = -1.0
    rot[np.arange(half) + half, np.arange(half)] = 1.0
    rot64 = np.ascontiguousarray(rot.T).astype(bf)
    rot128 = np.zeros((2 * ROPE_D, 2 * ROPE_D), dtype=f32)
    rot128[:ROPE_D, :ROPE_D] = rot.T
    rot128[ROPE_D:, ROPE_D:] = rot.T
    rot128 = rot128.astype(bf)

    ones_in = np.ones((128, 1), dtype=bf)
    xs = np.arange(896)[None, :] - 384
    ps = np.arange(128)[:, None]
    mask_strip = (xs >= ps).astype(bf)

    in_maps = []
    for c in range(NCORE):
        heads = list(range(HPC * c, HPC * (c + 1)))
        kbT = np.concatenate(
            [w_kvb_eff[h * 256 : h * 256 + NOPE_D, :] for h in heads], axis=0
        ).T  # [512 latent, 512]
        wkb_arr = np.zeros((128, KVKT * HPC * 128), dtype=f32)
        for k in range(KVKT):
            for h in range(HPC):
                blk = kbT[k * 128 : (k + 1) * 128, h * 128 : (h + 1) * 128]
                wkb_arr[:, (k * HPC + h) * 128 : (k * HPC + h + 1) * 128] = blk
        vbT = np.concatenate(
            [w_kvb_eff[h * 256 + NOPE_D : (h + 1) * 256, :] for h in heads], axis=0
        ).T  # [512 latent, 512 vdims]
        wvb_arr = np.zeros((128, KVKT * HPC * 128), dtype=f32)
        for k in range(KVKT):
            wvb_arr[:, k * 512 : (k + 1) * 512] = vbT[k * 128 : (k + 1) * 128, :]

        woT = w_o[:, c * HPC * V_D : (c + 1) * HPC * V_D].T  # [512, 4096] lhsT
        wo_arr = np.zeros((128, HPC * HIDDEN), dtype=f32)
        for h in range(HPC):
            wo_arr[:, h * HIDDEN : (h + 1) * HIDDEN] = woT[h * 128 : (h + 1) * 128, :]

        ht_c = hidden_T[:, c * SC : (c + 1) * SC]  # [4096, 256]
        ht_arr = np.ascontiguousarray(
            ht_c.reshape(HKT, 128, SC).transpose(1, 0, 2).reshape(128, HKT * SC)
        )
        cos_c = cos_T[:, c * SC : (c + 1) * SC]
        sin_c = sin_T[:, c * SC : (c + 1) * SC]

        in_maps.append(
            {
                "ht_in": ht_arr.astype(bf),
                "wkva_t": wkva_t,
                "wkr_t": wkr_t,
                "wqa_t": wqa_t,
                "wqb_t": wqb_t,
                "wkb_t": wkb_arr.astype(bf),
                "wvb_t": wvb_arr.astype(bf),
                "wo_t": wo_arr.astype(bf),
                "cos_kc": np.ascontiguousarray(cos_c).astype(bf),
                "sin_kc": np.ascontiguousarray(sin_c).astype(bf),
                "cos_q": np.ascontiguousarray(np.concatenate([cos_c, cos_c], 0)).astype(bf),
                "sin_q": np.ascontiguousarray(np.concatenate([sin_c, sin_c], 0)).astype(bf),
                "rot64": rot64,
                "rot128": rot128,
                "mask_strip": mask_strip,
                "ones_bf": ones_in,
                "eps_in": np.full((1, 1), EPS, dtype=f32),
            }
        )
    return in_maps


_NC_CACHE = {}


def _get_nc():
    if "nc" not in _NC_CACHE:
        _NC_CACHE["nc"] = build()
    return _NC_CACHE["nc"]


def run(inputs):
    nc = _get_nc()
    in_maps = make_in_maps(**inputs)
    res = run_bass_kernel_spmd(nc, in_maps, core_ids=list(range(NCORE)))
    return res


def kernel(**inputs) -> np.ndarray:
    res = run(inputs)
    acc = np.zeros((HIDDEN, S), dtype=np.float64)
    for c in range(NCORE):
        acc += res.results[c]["o_partial"]
    return np.ascontiguousarray(acc.T, dtype=np.float32).reshape(1, S, HIDDEN)


if __name__ == "__main__":
    import reference

    inputs = {k: np.asarray(v) for k, v in reference.setup_inputs().items()}
    out = kernel(**inputs)
    exp = np.asarray(reference.reference(**reference.setup_inputs()))
    rel = np.linalg.norm(out - exp) / np.linalg.norm(exp)
    print("Relative error:", rel)



# revision 31
# speedup vs baseline: 1.2121x; 1.2121x over previous
"""DeepSeek-V3 MLA attention (B=1, S=2048) on 8 TRN2 NeuronCores.

Sharding: low-rank down-projections are token-sharded (256 tokens per
core).  kv latents are RMS-normalized on the producer side and
AllGathered (512 latent + 64 roped k_rot rows; k_rot's rotate-half is
folded into a pre-rotated weight copy so rope is two muls and an add).
Each core computes the q up-projection for ALL 32 heads on its own
token chunk (the q RMS-norm scale is folded into the q_b outputs and
the rope tables); three AllToAlls (heads {0,1}, {2}, {3}) redistribute
q so each core ends up with its 4 heads over all 2048 tokens, arriving
at the rate attention consumes them.  Attention and o_proj are head-sharded; each
core returns a partial o_proj output which the host sums.

Attention: causal 512-query blocks; diagonal key-tiles compute only
the surviving trailing columns.  Softmax denominators are accumulated
on the vector engine (p_cum) with a single ones-matmul per (head,
query block), emitted after the next block's prefetch matmuls so the
PE never waits on the accumulate tail.  Score tiles are computed two
key-tiles ahead of the AV accumulation to hide the exp latency.

All activations/weights are bf16 (PSUM stays fp32).  Weights are
host-pretiled so every DMA moves >=512B contiguous runs; DMA issue
order is tuned against the single shared DMA-engine FIFO (the Tile
scheduler does not price collectives, so collective-dependent consumer
DMAs carry tile_wait_until hints to keep them out of the weight
stream's way).
"""

import numpy as np
import ml_dtypes

import concourse.bass as bass
import concourse.tile as tile
from concourse import bacc, mybir
from concourse.bass_utils import run_bass_kernel_spmd

F32 = mybir.dt.float32
BF16 = mybir.dt.bfloat16
NPBF16 = ml_dtypes.bfloat16
AF = mybir.ActivationFunctionType

HIDDEN = 4096
N_HEADS = 32
Q_LORA = 1536
KV_LORA = 512
ROPE_D = 64
NOPE_D = 128
V_D = 128
QH = NOPE_D + ROPE_D  # 192
EPS = 1e-6
SCALING = QH ** -0.5
S = 2048
NCORE = 8
SC = S // NCORE  # 256 tokens per core chunk
HPC = N_HEADS // NCORE  # 4 heads per core

QKT = Q_LORA // 128  # 12
KVKT = KV_LORA // 128  # 4
HKT = HIDDEN // 128  # 32
NGRP = 4  # q_b computation groups (1 local head each)
GT = 12  # m-tiles per group: 8 nope (per dest) + 4 packed rot pairs
KVR = KV_LORA + ROPE_D  # 576 rows in the kv gather
NA2A = 3  # AllToAlls: heads {0,1}, {2}, {3} - staged arrival
BLKS = [2 * QH, QH, QH]  # per-dest block rows of each AllToAll


def _qs(qb):
    return slice(qb * 512, (qb + 1) * 512)


def _kts(kt):
    return slice(kt * 128, (kt + 1) * 128)


def build():
    nc = bacc.Bacc(None, target_bir_lowering=False, num_devices=NCORE)

    def din(name, shape, dt=BF16):
        return nc.dram_tensor(name, shape, dt, kind="ExternalInput")

    # host-pretiled weights: row p of each [128, X] block is the SBUF
    # partition row (contiguous >=512B runs for every DMA)
    ht_in = din("ht_in", [128, HKT * SC])              # hidden own chunk
    wkva_t = din("wkva_t", [128, KVKT * HIDDEN])       # kv_a m-tiles
    wkr_t = din("wkr_t", [128, HKT * 2 * ROPE_D])      # k_rot raw|rot tiles
    wqa_t = din("wqa_t", [128, QKT * HIDDEN])          # q_a m-tiles
    wqb_t = din("wqb_t", [128, NGRP * GT * Q_LORA])    # q_b m-tiles (all heads)
    wkb_t = din("wkb_t", [128, KVKT * HPC * 128])      # k_nope up (own heads)
    wvb_t = din("wvb_t", [128, KVKT * HPC * 128])      # v up (own heads)
    wo_t = din("wo_t", [128, HPC * HIDDEN])            # o_proj (own heads)
    cos_kc = din("cos_kc", [ROPE_D, SC])               # own-chunk cos/sin
    sin_kc = din("sin_kc", [ROPE_D, SC])
    cos_q = din("cos_q", [2 * ROPE_D, SC])             # duplicated halves
    sin_q = din("sin_q", [2 * ROPE_D, SC])
    rot128 = din("rot128", [2 * ROPE_D, 2 * ROPE_D])
    mask_strip = din("mask_strip", [128, 896])
    ones_bf = din("ones_bf", [128, 1])
    eps_in = din("eps_in", [1, 1], F32)

    o_partial = nc.dram_tensor("o_partial", [HIDDEN, S], BF16, kind="ExternalOutput")

    ag_in_kv = nc.dram_tensor("ag_in_kv", [KVR, SC], BF16, kind="Internal")
    ag_out_kv = nc.dram_tensor(
        "ag_out_kv", [NCORE * KVR, SC], BF16, kind="Internal", addr_space="Shared"
    )
    a2a_in = [
        nc.dram_tensor(f"a2a_in{g}", [NCORE * BLKS[g], SC], BF16, kind="Internal")
        for g in range(NA2A)
    ]
    a2a_out = [
        nc.dram_tensor(f"a2a_out{g}", [NCORE * BLKS[g], SC], BF16, kind="Internal")
        for g in range(NA2A)
    ]

    with tile.TileContext(nc) as tc:
      with tc.tile_pool(name="root", bufs=1) as R:
        ones1 = R.tile([128, 1], BF16)
        nc.scalar.dma_start(ones1[:], ones_bf[:])
        epst = R.tile([1, 1], F32)
        nc.scalar.dma_start(epst[:], eps_in[:])
        # long-lived phase-2 tiles allocated up front so their DMAs can run
        # while phase 1 computes (they do not overlap phase-1 SBUF pools)
        wkb = R.tile([128, KVKT, HPC * 128], BF16)
        wvb = R.tile([128, KVKT, HPC * 128], BF16)
        wo_sb = R.tile([128, HPC, HIDDEN], BF16)
        mask_sb = R.tile([128, 896], BF16)
        lat = R.tile([128, KVKT, S], BF16)
        k_rot = R.tile([ROPE_D, S], BF16)
        with (
            tc.tile_pool(name="glob", bufs=1) as G,
            tc.tile_pool(name="ph1", bufs=1) as P1,
            tc.tile_pool(name="wstream", bufs=1) as WS,
            tc.tile_pool(name="ps_lat", bufs=5, space="PSUM") as PSL,
            tc.tile_pool(name="ps_acc", bufs=1, space="PSUM") as PSA,
        ):
            # ---- loads: first kv_a weight tile, then hidden, then the rest
            wkva_r = wkva_t[:].rearrange("p (m k n) -> p m k n", m=KVKT, k=HKT)
            ht = G.tile([128, HKT, SC], BF16)
            ht_r = ht_in[:].rearrange("p (k n) -> p k n", k=HKT)
            wkva_tiles = []
            wt0 = WS.tile([128, HKT, 128], BF16, tag="wkva", bufs=2)
            wkva_tiles.append(wt0)
            wt1 = WS.tile([128, HKT, 128], BF16, tag="wkva", bufs=2)
            wkva_tiles.append(wt1)
            nc.sync.dma_start(ht[:, 0:8, :], ht_r[:, 0:8])
            nc.sync.dma_start(wt0[:, 0:16, :], wkva_r[:, 0, 0:16])
            nc.sync.dma_start(ht[:, 8:16, :], ht_r[:, 8:16])
            nc.sync.dma_start(wt0[:, 16:32, :], wkva_r[:, 0, 16:32])
            nc.sync.dma_start(ht[:, 16:24, :], ht_r[:, 16:24])
            nc.sync.dma_start(ht[:, 24:32, :], ht_r[:, 24:32])
            nc.sync.dma_start(wt1[:, 0:16, :], wkva_r[:, 1, 0:16])
            nc.sync.dma_start(wt1[:, 16:32, :], wkva_r[:, 1, 16:32])
            cckc = P1.tile([ROPE_D, SC], BF16)
            sckc = P1.tile([ROPE_D, SC], BF16)
            nc.scalar.dma_start(cckc[:], cos_kc[:])
            nc.scalar.dma_start(sckc[:], sin_kc[:])
            r128 = P1.tile([2 * ROPE_D, 2 * ROPE_D], BF16)
            nc.scalar.dma_start(r128[:], rot128[:])
            ccq = P1.tile([2 * ROPE_D, SC], BF16)
            scq = P1.tile([2 * ROPE_D, SC], BF16)
            nc.scalar.dma_start(ccq[:], cos_q[:])
            nc.scalar.dma_start(scq[:], sin_q[:])

            # ================= phase 1a: kv latents (own chunk) =========
            # matmul to PSUM, deferred ssq row, producer-side RMS norm
            # folded into the PSUM->SBUF copies, then AllGather.
            kvraw = P1.tile([128, KVKT, SC], BF16)
            kv_ps = []
            sskv = PSA.tile([1, SC], F32, name="sskv")
            kv_sq = []
            def kv_a_mtile(m):
                ps = PSL.tile([128, SC], F32, tag="pslat")
                if m < 2:
                    wt = wkva_tiles[m]
                else:
                    wt = WS.tile([128, HKT, 128], BF16, tag="wkva", bufs=2)
                    nc.sync.dma_start(wt[:], wkva_r[:, m])
                for k in range(HKT):
                    nc.tensor.matmul(
                        ps[:], wt[:, k, :], ht[:, k, :], start=(k == 0), stop=(k == HKT - 1)
                    )
                kv_ps.append(ps)
                sq = WS.tile([128, SC], BF16, tag="lsq", bufs=3)
                nc.scalar.activation(sq[:], ps[:], AF.Square)
                kv_sq.append(sq)
                if m >= 1:  # deferred ssq matmul: PE never waits on Act
                    nc.tensor.matmul(
                        sskv[:], ones1[:], kv_sq[m - 1][:], start=(m == 1), stop=False
                    )
            for m in range(2):
                kv_a_mtile(m)
            # k_rot (rows 512:576 of ckv): raw and pre-rotated halves come
            # out of one matmul tile; rope is then two muls and an add.
            psr = PSL.tile([128, SC], F32, tag="psrot", bufs=1)
            wtr = WS.tile([128, HKT, 2 * ROPE_D], BF16, tag="wkr", bufs=1)
            nc.sync.dma_start(wtr[:], wkr_t[:].rearrange("p (k n) -> p k n", k=HKT))
            for k in range(HKT):
                nc.tensor.matmul(
                    psr[:], wtr[:, k, :], ht[:, k, :], start=(k == 0), stop=(k == HKT - 1)
                )
            ktmp = P1.tile([ROPE_D, SC], BF16)
            nc.vector.tensor_mul(ktmp[:], psr[ROPE_D : 2 * ROPE_D, :], sckc[:])
            kfin = P1.tile([ROPE_D, SC], BF16)
            nc.vector.tensor_mul(kfin[:], psr[0:ROPE_D, :], cckc[:])
            nc.vector.tensor_add(kfin[:], kfin[:], ktmp[:])
            nc.scalar.dma_start(ag_in_kv[KV_LORA:KVR, :], kfin[:])

            for m in range(2, KVKT):
                kv_a_mtile(m)
            nc.tensor.matmul(sskv[:], ones1[:], kv_sq[-1][:], start=False, stop=True)
            sqkv = P1.tile([1, SC], F32)
            nc.scalar.activation(sqkv[:], sskv[:], AF.Sqrt, scale=1.0 / KV_LORA, bias=epst[:])
            rkv = P1.tile([1, SC], F32)
            nc.vector.reciprocal(rkv[:], sqkv[:])
            rkv_bc = P1.tile([128, SC], F32)
            nc.gpsimd.partition_broadcast(rkv_bc[:], rkv[:])
            for m in range(KVKT):
                nc.vector.tensor_mul(kvraw[:, m, :], kv_ps[m][:], rkv_bc[:])
            nc.scalar.dma_start(
                ag_in_kv[0:KV_LORA, :].rearrange("(m p) n -> p m n", p=128), kvraw[:]
            )

            nc.gpsimd.collective_compute(
                "AllGather",
                mybir.AluOpType.bypass,
                replica_groups=[list(range(NCORE))],
                ins=[ag_in_kv[:]],
                outs=[ag_out_kv[:]],
            )

            # ================= phase 1b: q latents (own chunk) ==========
            qraw = P1.tile([128, QKT, SC], BF16)
            ssq = PSA.tile([1, SC], F32, name="ssq")
            wqa_r = wqa_t[:].rearrange("p (m k n) -> p m k n", m=QKT, k=HKT)
            q_sq = []
            for m in range(QKT):
                ps = PSL.tile([128, SC], F32, tag="pslat")
                wt = WS.tile([128, HKT, 128], BF16, tag="wqa", bufs=3)
                if m in (2, 3):
                    # halves: the tiny ag_in DMAs racing on the DMA FIFO
                    # then wait <=1.5us instead of behind a whole tile
                    nc.sync.dma_start(wt[:, 0:16, :], wqa_r[:, m, 0:16])
                    nc.sync.dma_start(wt[:, 16:32, :], wqa_r[:, m, 16:32])
                else:
                    nc.sync.dma_start(wt[:], wqa_r[:, m])
                for k in range(HKT):
                    nc.tensor.matmul(
                        ps[:], wt[:, k, :], ht[:, k, :], start=(k == 0), stop=(k == HKT - 1)
                    )
                nc.vector.tensor_copy(qraw[:, m, :], ps[:])
                sq = WS.tile([128, SC], BF16, tag="lsq", bufs=3)
                nc.scalar.activation(sq[:], ps[:], AF.Square)
                q_sq.append(sq)
                if m >= 1:
                    nc.tensor.matmul(
                        ssq[:], ones1[:], q_sq[m - 1][:], start=(m == 1), stop=False
                    )
            nc.tensor.matmul(ssq[:], ones1[:], q_sq[-1][:], start=False, stop=True)

            nc.sync.dma_start(wkb[:], wkb_t[:].rearrange("p (k n) -> p k n", k=KVKT))
            nc.sync.dma_start(wvb[:], wvb_t[:].rearrange("p (k n) -> p k n", k=KVKT))
            nc.sync.dma_start(mask_sb[:], mask_strip[:])

            sq1 = P1.tile([1, SC], F32)
            nc.scalar.activation(sq1[:], ssq[:], AF.Sqrt, scale=1.0 / Q_LORA, bias=epst[:])
            rq = P1.tile([1, SC], F32)
            nc.vector.reciprocal(rq[:], sq1[:])
            rq_bc = P1.tile([128, SC], F32)
            nc.gpsimd.partition_broadcast(rq_bc[:], rq[:])
            # rmsnorm scale commutes with q_b: scale outputs, not the latent.
            # rope tables pre-multiplied by rq so roped tiles pick it up too.
            ccq_rq = P1.tile([2 * ROPE_D, SC], BF16)
            scq_rq = P1.tile([2 * ROPE_D, SC], BF16)
            nc.vector.tensor_mul(ccq_rq[:], ccq[:], rq_bc[:])
            nc.vector.tensor_mul(scq_rq[:], scq[:], rq_bc[:])

            # ================= phase 1c: q_b all heads (own chunk) ======
            # m-tiles per group g (local head g of each dest): 8 nope
            # tiles (dest-major) then 4 rot tiles packing dests (2j|2j+1).
            # Groups 2j,2j+1 ship in AllToAll j (per-dest 384-row block:
            # [nope(2j) 128 | rot(2j) 64 | nope(2j+1) 128 | rot(2j+1) 64]).
            wqb_r = wqb_t[:].rearrange("p (t k n) -> p t k n", t=NGRP * GT, k=QKT)
            qown = [G.tile([128, GT, SC], BF16, name=f"qown{g}") for g in range(NGRP)]
            ag_rcn = ag_out_kv[:].rearrange("(c r) n -> r c n", c=NCORE)
            for g in range(NGRP):
                if g == 3:
                    # kv latents + k_rot loads: ACT queue, held back in
                    # scheduler time (the scheduler does not price
                    # collectives, so it would hoist these to the front of
                    # the FIFO and head-block phase-1 ACT work)
                    with tc.tile_wait_until(float(os.environ.get("KLAT", "0.112"))):
                        for m in range(KVKT):
                            nc.scalar.dma_start(
                                lat[:, m, :].rearrange("p (c n) -> p c n", c=NCORE),
                                ag_rcn[m * 128 : (m + 1) * 128],
                            )
                        nc.scalar.dma_start(
                            k_rot[:].rearrange("p (c n) -> p c n", c=NCORE),
                            ag_rcn[KV_LORA:KVR],
                        )
                for mt in range(GT):
                    ps = PSL.tile([128, SC], F32, tag="pslat")
                    wt = WS.tile([128, QKT, 128], BF16, tag="wqb", bufs=10)
                    nc.sync.dma_start(wt[:], wqb_r[:, g * GT + mt])
                    for k in range(QKT):
                        nc.tensor.matmul(
                            ps[:], wt[:, k, :], qraw[:, k, :], start=(k == 0), stop=(k == QKT - 1)
                        )
                    if mt < 8:
                        nc.vector.tensor_mul(qown[g][:, mt, :], ps[:], rq_bc[:])
                    elif mt % 2 == 0:
                        nc.vector.tensor_copy(qown[g][:, mt, :], ps[:])
                    else:
                        nc.scalar.copy(qown[g][:, mt, :], ps[:])
                # rope the 4 rot-pair tiles (mt = 8..11)
                for j in range(4):
                    rtile = qown[g][:, 8 + j, :]
                    rp2 = PSL.tile([128, SC], F32, tag="pslat")
                    nc.tensor.matmul(rp2[:], r128[:], rtile, start=True, stop=True)
                    rtmp = WS.tile([128, SC], BF16, tag="rtmp", bufs=2)
                    nc.vector.tensor_mul(rtmp[:], rp2[:], scq_rq[:])
                    nc.vector.tensor_mul(rtile, rtile, ccq_rq[:])
                    nc.vector.tensor_add(rtile, rtile, rtmp[:])
                # ship into AllToAll jj at sub-block offset p_off: nope
                # tile d -> per-dest rows p_off..+128; rot tile j halves ->
                # within the 2-dest rot superblock at p_off+128 and
                # blk+p_off+128
                jj = 0 if g < 2 else g - 1
                p_off = 192 * g if g < 2 else 0
                blk = BLKS[jj]
                a2a_nope = a2a_in[jj][:].rearrange("(d b) n -> b d n", d=NCORE)
                nc.scalar.dma_start(
                    a2a_nope[p_off : p_off + 128], qown[g][:, 0:8, :]
                )
                a2a_rot = a2a_in[jj][:].rearrange("(j b) n -> b j n", j=4)
                nc.scalar.dma_start(
                    a2a_rot[p_off + 128 : p_off + 192], qown[g][0:64, 8:12, :]
                )
                nc.scalar.dma_start(
                    a2a_rot[blk + p_off + 128 : blk + p_off + 192],
                    qown[g][64:128, 8:12, :],
                )
                if g >= 1:
                    nc.gpsimd.collective_compute(
                        "AllToAll",
                        mybir.AluOpType.bypass,
                        replica_groups=[list(range(NCORE))],
                        ins=[a2a_in[jj][:]],
                        outs=[a2a_out[jj][:]],
                    )



        # ================= phase 2: head-sharded attention ==============
        with tc.tile_pool(name="p2", bufs=1) as P2:
            # kv_b: kn (4 heads) and v (all tokens)
            kn = P2.tile([128, HPC, S], BF16)
            vp = P2.tile([128, 16, HPC * 128], BF16)
            with (
                tc.tile_pool(name="ps2", bufs=4, space="PSUM") as PSK,
                tc.tile_pool(name="ps2o", bufs=2, space="PSUM") as PAO,
                tc.tile_pool(name="ps2s", bufs=2, space="PSUM") as PSS,
            ):
                for h in range(HPC):
                    for cb in range(4):
                        pk = PSK.tile([128, 512], F32, tag="sps")
                        for k in range(KVKT):
                            nc.tensor.matmul(
                                pk[:],
                                wkb[:, k, h * 128 : (h + 1) * 128],
                                lat[:, k, _qs(cb)],
                                start=(k == 0),
                                stop=(k == KVKT - 1),
                            )
                        if cb % 2 == 0:
                            nc.scalar.copy(kn[:, h, _qs(cb)], pk[:])
                        else:
                            nc.vector.tensor_copy(kn[:, h, _qs(cb)], pk[:])
                for tt in range(16):
                    pv = PSK.tile([128, 512], F32, tag="sps")
                    for k in range(KVKT):
                        nc.tensor.matmul(
                            pv[:],
                            lat[:, k, _kts(tt)],
                            wvb[:, k, :],
                            start=(k == 0),
                            stop=(k == KVKT - 1),
                        )
                    if tt % 2 == 0:
                        nc.scalar.copy(vp[:, tt, :], pv[:])
                    else:
                        nc.vector.tensor_copy(vp[:, tt, :], pv[:])

                o_sb = P2.tile([128, HPC, S], BF16)

                # softmax normalize for a finished query block; emitted after
                # the NEXT block's prefetch matmuls so the PE never waits on
                # the DVE accumulate tail
                pending_norm = []

                def flush_norm():
                    while pending_norm:
                        pending_norm.pop(0)()

                def defer_norm(h, qb, ops, p_cum):
                    def go():
                        ssum = PSS.tile([1, 512], F32, tag="ssum")
                        nc.tensor.matmul(
                            ssum[:], ones1[:], p_cum[:], start=True, stop=True
                        )
                        rec = P2.tile([1, 512], F32, tag="rec", bufs=2)
                        nc.vector.reciprocal(rec[:], ssum[:])
                        rb = P2.tile([128, 512], F32, tag="rb", bufs=2)
                        nc.gpsimd.partition_broadcast(rb[:], rec[:])
                        nc.vector.tensor_mul(o_sb[:, h, _qs(qb)], ops[:], rb[:])

                    pending_norm.append(go)

                # attention: one flat (head, qb, kt) stream with a rolling
                # prefetch that survives qb/head boundaries, so the exp
                # pipeline never drains and refills
                head_state = {}

                def head_init(h):
                    jj = 0 if h < 2 else h - 1
                    p_off = 192 * h if h < 2 else 0
                    a2a_r = a2a_out[jj][:].rearrange("(c b) n -> b c n", c=NCORE)
                    qn = P2.tile([128, S], BF16, tag="qn", bufs=2)
                    qr = P2.tile([ROPE_D, S], BF16, tag="qr", bufs=2)
                    # first query block lands first: the head's first score
                    # matmul waits ~1.5us less after the AllToAll
                    for piece in (slice(0, 2), slice(2, NCORE)):
                        nc.sync.dma_start(
                            qn[:, piece.start * SC : piece.stop * SC].rearrange(
                                "p (c n) -> p c n", c=piece.stop - piece.start
                            ),
                            a2a_r[p_off : p_off + 128, piece],
                        )
                        nc.sync.dma_start(
                            qr[:, piece.start * SC : piece.stop * SC].rearrange(
                                "p (c n) -> p c n", c=piece.stop - piece.start
                            ),
                            a2a_r[p_off + 128 : p_off + 192, piece],
                        )
                    if h == 1:
                        # o_proj weights ride behind h0/h1 q loads: DMA
                        # engines are quiet here, well before o_proj needs them
                        wo_r = wo_t[:].rearrange("p (h n) -> p h n", h=HPC)
                        for wh in range(HPC):
                            nc.sync.dma_start(wo_sb[:, wh, :], wo_r[:, wh])
                    head_state[h] = (qn, qr)

                def _off(qb, kt):
                    v = kt - 4 * qb
                    return 128 * v if v >= 1 else 0

                items = [
                    (h, qb, kt)
                    for h in range(HPC)
                    for qb in range(4)
                    for kt in range(4 * (qb + 1))
                ]

                def mk_sps(i):
                    h, qb, kt = items[i]
                    if h not in head_state:
                        head_init(h)
                    qn, qr = head_state[h]
                    o = _off(qb, kt)
                    sps = PSK.tile([128, 512], F32, tag="sps")
                    nc.tensor.matmul(
                        sps[:, o:512],
                        kn[:, h, _kts(kt)],
                        qn[:, qb * 512 + o : (qb + 1) * 512],
                        start=True,
                        stop=False,
                    )
                    nc.tensor.matmul(
                        sps[:, o:512],
                        k_rot[:, _kts(kt)],
                        qr[:, qb * 512 + o : (qb + 1) * 512],
                        start=False,
                        stop=True,
                    )
                    return sps

                PF = 2  # score tiles computed ahead of AV
                queue = [mk_sps(i) for i in range(PF)]
                blk_state = {}
                prev_blk = None
                for i, (h, qb, kt) in enumerate(items):
                    if i + PF < len(items):
                        queue.append(mk_sps(i + PF))
                    sps = queue.pop(0)
                    if (h, qb) != prev_blk:
                        flush_norm()
                        prev_blk = (h, qb)
                    kt_max = 4 * (qb + 1)
                    o = _off(qb, kt)
                    w = 512 - o
                    if kt == 0:
                        ops = PAO.tile([128, 512], F32, tag="ops")
                        p_cum = P2.tile([128, 512], BF16, tag="pcum", bufs=2)
                        blk_state[(h, qb)] = (ops, p_cum)
                        pt = p_cum[:]
                    else:
                        ops, p_cum = blk_state[(h, qb)]
                        ptt = P2.tile([128, 512], BF16, tag="pt", bufs=4)
                        pt = ptt[:, 0:w]
                    nc.scalar.activation(pt, sps[:, o:512], AF.Exp, scale=SCALING)
                    v_idx = kt - 4 * qb
                    if v_idx >= 0:
                        nc.vector.tensor_mul(pt, pt, mask_sb[:, 384 : 384 + w])
                    if kt > 0:
                        nc.vector.tensor_add(p_cum[:, o:512], p_cum[:, o:512], pt)
                    nc.tensor.matmul(
                        ops[:, o:512],
                        vp[:, kt, h * 128 : (h + 1) * 128],
                        pt,
                        start=(kt == 0),
                        stop=(kt == kt_max - 1),
                    )
                    if kt == kt_max - 1:
                        defer_norm(h, qb, ops, p_cum)
                        del blk_state[(h, qb)]
                flush_norm()

                # ---------------- o_proj --------------------------------
                for m in range(HIDDEN // 128):
                    ost = P2.tile([128, 4, 512], BF16, tag="ost", bufs=2)
                    for qb in range(4):
                        po = PSK.tile([128, 512], F32, tag="sps")
                        for hh in range(HPC):
                            nc.tensor.matmul(
                                po[:],
                                wo_sb[:, hh, m * 128 : (m + 1) * 128],
                                o_sb[:, hh, _qs(qb)],
                                start=(hh == 0),
                                stop=(hh == HPC - 1),
                            )
                        if qb % 2 == 0:
                            nc.vector.tensor_copy(ost[:, qb, :], po[:])
                        else:
                            nc.scalar.copy(ost[:, qb, :], po[:])
                    nc.sync.dma_start(
                        o_partial[m * 128 : (m + 1) * 128, :],
                        ost[:].rearrange("p a b -> p (a b)"),
                    )

    nc.compile()
    return nc


def _tile_lhsT(wT, n_k, n_m):
    """wT: [K, M] (lhsT layout, K contracting).  Returns pretiled
    [128, n_m*n_k*128]: A[p, m, k, c] = wT[k*128+p, m*128+c]."""
    K, M = wT.shape
    assert K == n_k * 128 and M == n_m * 128
    A = wT.reshape(n_k, 128, n_m, 128).transpose(1, 2, 0, 3)  # [p, m, k, c]
    return np.ascontiguousarray(A.reshape(128, n_m * n_k * 128))


def make_in_maps(
    hidden_states, cos, sin, w_q_a, q_a_ln_w, w_q_b, w_kv_a, kv_a_ln_w, w_kv_b, w_o
):
    f32 = np.float32
    bf = NPBF16
    hidden_T = np.asarray(hidden_states, dtype=f32)[0].T  # [4096, 2048]
    cos_T = np.asarray(cos, dtype=f32)[0].T  # [64, 2048]
    sin_T = np.asarray(sin, dtype=f32)[0].T

    w_q_a = np.asarray(w_q_a, dtype=f32)
    w_kv_a = np.asarray(w_kv_a, dtype=f32)
    w_qb_eff = np.asarray(w_q_b, dtype=f32) * np.asarray(q_a_ln_w, dtype=f32)[None, :]
    w_kvb_eff = np.asarray(w_kv_b, dtype=f32) * np.asarray(kv_a_ln_w, dtype=f32)[None, :]
    w_o = np.asarray(w_o, dtype=f32)

    wqa_t = _tile_lhsT(w_q_a.T.astype(bf), HKT, QKT)
    wkva_t = _tile_lhsT(w_kv_a[:KV_LORA].T.astype(bf), HKT, KVKT)
    rot_m = np.zeros((ROPE_D, ROPE_D), dtype=f32)
    halfd = ROPE_D // 2
    rot_m[np.arange(halfd), np.arange(halfd) + halfd] = -1.0
    rot_m[np.arange(halfd) + halfd, np.arange(halfd)] = 1.0
    wkr = w_kv_a[KV_LORA:].T  # [4096, 64]
    wkr2 = np.concatenate([wkr, wkr @ rot_m.T], axis=1).astype(bf)  # raw | rot
    wkr_t = np.ascontiguousarray(
        wkr2.reshape(HKT, 128, 2 * ROPE_D).transpose(1, 0, 2).reshape(128, HKT * 2 * ROPE_D)
    )

    # q_b m-tiles: group g (local head g): 8 nope tiles (dest-major), then
    # 4 rot tiles packing dests (2j | 2j+1)
    wqbT = w_qb_eff.T.astype(bf)  # [1536, 6144]
    mtiles = []
    for g in range(NGRP):
        for d in range(NCORE):
            hd = 4 * d + g
            mtiles.append(wqbT[:, hd * QH : hd * QH + NOPE_D])
        for j in range(4):
            h0 = 4 * (2 * j) + g
            h1 = 4 * (2 * j + 1) + g
            rp = np.concatenate(
                [
                    wqbT[:, h0 * QH + NOPE_D : (h0 + 1) * QH],
                    wqbT[:, h1 * QH + NOPE_D : (h1 + 1) * QH],
                ],
                axis=1,
            )
            mtiles.append(rp)
    wqb_all = np.concatenate(mtiles, axis=1)  # [1536, 48*128]
    wqb_t = _tile_lhsT(wqb_all, QKT, NGRP * GT)

    rot128 = np.zeros((2 * ROPE_D, 2 * ROPE_D), dtype=f32)
    rot128[:ROPE_D, :ROPE_D] = rot_m.T
    rot128[ROPE_D:, ROPE_D:] = rot_m.T
    rot128 = rot128.astype(bf)

    ones_in = np.ones((128, 1), dtype=bf)
    xs = np.arange(896)[None, :] - 384
    ps = np.arange(128)[:, None]
    mask_strip = (xs >= ps).astype(bf)

    in_maps = []
    for c in range(NCORE):
        heads = list(range(HPC * c, HPC * (c + 1)))
        kbT = np.concatenate(
            [w_kvb_eff[h * 256 : h * 256 + NOPE_D, :] for h in heads], axis=0
        ).T  # [512 latent, 512]
        wkb_arr = np.zeros((128, KVKT * HPC * 128), dtype=f32)
        for k in range(KVKT):
            for h in range(HPC):
                blk = kbT[k * 128 : (k + 1) * 128, h * 128 : (h + 1) * 128]
                wkb_arr[:, (k * HPC + h) * 128 : (k * HPC + h + 1) * 128] = blk
        vbT = np.concatenate(
            [w_kvb_eff[h * 256 + NOPE_D : (h + 1) * 256, :] for h in heads], axis=0
        ).T  # [512 latent, 512 vdims]
        wvb_arr = np.zeros((128, KVKT * HPC * 128), dtype=f32)
        for k in range(KVKT):
            wvb_arr[:, k * 512 : (k + 1) * 512] = vbT[k * 128 : (k + 1) * 128, :]

        woT = w_o[:, c * HPC * V_D : (c + 1) * HPC * V_D].T  # [512, 4096] lhsT
        wo_arr = np.zeros((128, HPC * HIDDEN), dtype=f32)
        for h in range(HPC):
            wo_arr[:, h * HIDDEN : (h + 1) * HIDDEN] = woT[h * 128 : (h + 1) * 128, :]

        ht_c = hidden_T[:, c * SC : (c + 1) * SC]  # [4096, 256]
        ht_arr = np.ascontiguousarray(
            ht_c.reshape(HKT, 128, SC).transpose(1, 0, 2).reshape(128, HKT * SC)
        )
        cos_c = cos_T[:, c * SC : (c + 1) * SC]
        sin_c = sin_T[:, c * SC : (c + 1) * SC]

        in_maps.append(
            {
                "ht_in": ht_arr.astype(bf),
                "wkva_t": wkva_t,
                "wkr_t": wkr_t,
                "wqa_t": wqa_t,
                "wqb_t": wqb_t,
                "wkb_t": wkb_arr.astype(bf),
                "wvb_t": wvb_arr.astype(bf),
                "wo_t": wo_arr.astype(bf),
                "cos_kc": np.ascontiguousarray(cos_c).astype(bf),
                "sin_kc": np.ascontiguousarray(sin_c).astype(bf),
                "cos_q": np.ascontiguousarray(np.concatenate([cos_c, cos_c], 0)).astype(bf),
                "sin_q": np.ascontiguousarray(np.concatenate([sin_c, sin_c], 0)).astype(bf),
                "rot128": rot128,
                "mask_strip": mask_strip,
                "ones_bf": ones_in,
                "eps_in": np.full((1, 1), EPS, dtype=f32),
            }
        )
    return in_maps


_NC_CACHE = {}


def _get_nc():
    if "nc" not in _NC_CACHE:
        _NC_CACHE["nc"] = build()
    return _NC_CACHE["nc"]


def run(inputs):
    nc = _get_nc()
    in_maps = make_in_maps(**inputs)
    res = run_bass_kernel_spmd(nc, in_maps, core_ids=list(range(NCORE)))
    return res


def kernel(**inputs) -> np.ndarray:
    res = run(inputs)
    acc = np.zeros((HIDDEN, S), dtype=np.float64)
    for c in range(NCORE):
        acc += res.results[c]["o_partial"]
    return np.ascontiguousarray(acc.T, dtype=np.float32).reshape(1, S, HIDDEN)


if __name__ == "__main__":
    import reference

    inputs = {k: np.asarray(v) for k, v in reference.setup_inputs().items()}
    out = kernel(**inputs)
    exp = np.asarray(reference.reference(**reference.setup_inputs()))
    rel = np.linalg.norm(out - exp) / np.linalg.norm(exp)
    print("Relative error:", rel)
